# revision 6
# baseline (speedup 1.0000x reference)
"""Trainium2 Bass kernel for nn_IterativeStructureRefiner (v4, bf16, full-row strips).

Math (validated vs reference: fp32 9e-8, bf16-quantized ~3e-3 l2rel):
  Rank-3 orientation factorization of the 8-neighbor affinity:
    num = oxx*S1 + oyy*S2 + oxy*S3,  S1 = Bm@hx, S2 = T0@cs + H0@hx,
    S3 = A0@csL - A0@csR   (cs = continuity*s, hx = csL+csR; vertical
    stencils are banded 128x128 stationaries on the TensorEngine).
  den = same stencils on continuity (iteration-invariant) -> folded with
  the uncertainty gate into precomputed coefficient maps:
    Cxx = 0.25*g*r*oxx (etc.), G9 = 0.25*g/9, r = 1/(den+eps).
  Smooth term in one PE pass: Q9 = T0I@sL + (T0I-9I)@sC + T0I@sR
    = 9*(box3x3(s) - s), so  s' = 0.75*s + G9*Q9 + Cxx*S1+Cyy*S2+Cxy*S3.
  The reference's final clip is provably inactive.

Geometry: 9 full-width row strips [128 part x 1024 cols] per image. Columns
have NO inter-patch halos - the only column pads are out-of-image zeros
(the reference zero-pads there), so stencil outputs are computed on exactly
1024 = 2x512 columns: every matmul chunk is 512 wide (one PSUM bank), no
tail matmuls. Rows keep the 6-row shrink-halo (116-row interior per strip).

All loop tensors bf16 (DVE 2x packing; 4B-aligned windows). PSUM rotates
two tags x two buffers. S1/S2 drain to bf16 on the Scalar engine; m3/t2
read PSUM directly on Vector; GpSimd takes cs + two adds per iteration.

Sharding: pure data-parallel, one batch image per NeuronCore (B=8).
Inputs are read once, output written once; all 6 iterations run locally.
"""

import numpy as np

H = W = 1024
TILE_W = 1040          # tile col t <-> image col t - 8 ; pads [0:8) [1032:1040)
W0, W1 = 8, 1032       # working window = the full image row (1024 cols)
PW = W1 - W0           # 1024
ROWS_OUT = 116
NUM_ITERS = 6
EPS = 1e-6

_CACHE = {}


def _build_bass():
    import concourse.bacc as bacc
    import concourse.mybir as mybir
    from concourse.tile import TileContext

    fp32 = mybir.dt.float32
    bf16 = mybir.dt.bfloat16
    Alu = mybir.AluOpType
    Act = mybir.ActivationFunctionType

    nc = bacc.Bacc("TRN2", debug=False)

    cen_d = nc.dram_tensor("center", [H, W], fp32, kind="ExternalInput")
    con_d = nc.dram_tensor("continuity", [H, W], fp32, kind="ExternalInput")
    ori_d = nc.dram_tensor("orientation", [2, H, W], fp32, kind="ExternalInput")
    unc_d = nc.dram_tensor("uncertainty", [H, W], fp32, kind="ExternalInput")
    out_d = nc.dram_tensor("out", [H, W], fp32, kind="ExternalOutput")

    k = np.arange(128)
    T0 = (np.abs(k[:, None] - k[None, :]) == 1).astype(np.float32)
    Bm = np.eye(128, dtype=np.float32) + 0.5 * T0
    H0 = 0.5 * T0
    A0 = ((k[:, None] == k[None, :] - 1).astype(np.float32)
          - (k[:, None] == k[None, :] + 1).astype(np.float32))
    A0m = -A0
    T0I = np.eye(128, dtype=np.float32) + T0
    T0I9 = T0 - 8.0 * np.eye(128, dtype=np.float32)
    ST = [Bm, T0, H0, A0, A0m, T0I, T0I9]
    st_drams = [nc.inline_tensor(m, name=f"st_{i}") for i, m in enumerate(ST)]
    botmask_np = (np.arange(128) < 102).astype(np.float32)[:, None]
    bot_dram = nc.inline_tensor(botmask_np, name="botmask")

    row_panels = []
    for r0 in range(0, H, ROWS_OUT):
        row_panels.append((r0, min(r0 + ROWS_OUT, H)))

    # psum col j <-> tile col j + W0 ; chunks are exactly one bank each
    CH = ((0, 512), (512, 1024))

    with TileContext(nc) as tc:
        with (
            tc.tile_pool(name="consts", bufs=1) as cpool,
            tc.tile_pool(name="inp", bufs=2) as ipool,
            tc.tile_pool(name="pre", bufs=2) as ppool,
            tc.tile_pool(name="scr", bufs=2) as spool,
            tc.tile_pool(name="fpre", bufs=1) as fpool,
            tc.tile_pool(name="stg", bufs=1) as gpool,
            tc.tile_pool(name="outp", bufs=1) as opool,
            tc.tile_pool(name="psum", bufs=2, space="PSUM") as qpool,
        ):
            st_b = []
            for i, d in enumerate(st_drams):
                tf = gpool.tile([128, 128], fp32, tag=f"stf{i}")
                nc.sync.dma_start(out=tf[:], in_=d[:, :])
                tb = cpool.tile([128, 128], bf16, tag=f"st{i}")
                nc.scalar.copy(tb[:], tf[:])
                st_b.append(tb)
            tBm, tT0, tH0, tA0, tA0m, tT0I, tT0I9 = st_b
            botmask = cpool.tile([128, 1], fp32, tag="botmask")
            nc.sync.dma_start(out=botmask[:], in_=bot_dram[:, :])

            # persistent bf16 ping-pong s tiles; pads zeroed once
            s_ab = []
            for nm in ("s_a", "s_b"):
                t = cpool.tile([128, TILE_W], bf16, tag=nm)
                nc.vector.memset(t[:, 0:W0], 0.0)
                nc.vector.memset(t[:, W1:TILE_W], 0.0)
                s_ab.append(t)

            def stencils(src, hxt, q1t, q2t, q3t):
                """q1=Bm@hx, q2=T0@src+H0@hx, q3=A0@srcL-A0@srcR.
                hxt[k] holds the hx value of tile col k+1 (shift-stored).
                Grouped per stationary to minimize LDWEIGHTS reloads."""
                for lo, hi in CH:
                    nc.tensor.matmul(q1t[:, lo:hi], tBm[:],
                                     hxt[:, W0 - 1 + lo:W0 - 1 + hi],
                                     start=True, stop=True)
                for lo, hi in CH:
                    nc.tensor.matmul(q2t[:, lo:hi], tT0[:],
                                     src[:, W0 + lo:W0 + hi],
                                     start=True, stop=False)
                for lo, hi in CH:
                    nc.tensor.matmul(q2t[:, lo:hi], tH0[:],
                                     hxt[:, W0 - 1 + lo:W0 - 1 + hi],
                                     start=False, stop=True)
                for lo, hi in CH:
                    nc.tensor.matmul(q3t[:, lo:hi], tA0[:],
                                     src[:, W0 - 1 + lo:W0 - 1 + hi],
                                     start=True, stop=False)
                for lo, hi in CH:
                    nc.tensor.matmul(q3t[:, lo:hi], tA0m[:],
                                     src[:, W0 + 1 + lo:W0 + 1 + hi],
                                     start=False, stop=True)

            def qsmooth(s_cur, q9t):
                for off in (W0 - 1, W0 + 1):
                    for lo, hi in CH:
                        nc.tensor.matmul(q9t[:, lo:hi], tT0I[:],
                                         s_cur[:, off + lo:off + hi],
                                         start=(off == W0 - 1), stop=False)
                for lo, hi in CH:
                    nc.tensor.matmul(q9t[:, lo:hi], tT0I9[:],
                                     s_cur[:, W0 + lo:W0 + hi],
                                     start=False, stop=True)

            WSL = slice(W0, W1)

            class Panel:
                pass

            def make_panel(r0, r1):
                P = Panel()
                P.r0, P.r1 = r0, r1
                P.row_lo = max(r0 - 6, 0)
                P.row_hi = min(r0 + 122, H)
                P.p_lo = P.row_lo - (r0 - 6)
                P.p_hi = P.row_hi - (r0 - 6)
                return P

            def emit_load(P):
                def load(src_ap, tag):
                    t = ipool.tile([128, TILE_W], fp32, tag=tag)
                    nc.gpsimd.memset(t[:, 0:W0], 0.0)
                    nc.gpsimd.memset(t[:, W1:TILE_W], 0.0)
                    if P.p_lo > 0:
                        nc.gpsimd.memset(t[0:P.p_lo, W0:W1], 0.0)
                    if P.p_hi < 128:
                        aligned_lo = (P.p_hi // 32) * 32
                        nc.gpsimd.memset(t[aligned_lo:128, W0:W1], 0.0)
                    nc.sync.dma_start(
                        out=t[P.p_lo:P.p_hi, WSL],
                        in_=src_ap[P.row_lo:P.row_hi, :])
                    return t

                P.t_cen = load(cen_d, "cen")
                P.t_con = load(con_d, "con")
                P.t_ox = load(ori_d[0], "ox")
                P.t_oy = load(ori_d[1], "oy")
                P.t_unc = load(unc_d, "unc")

            def emit_pre(P):
                P.cont_b = ppool.tile([128, TILE_W], bf16, tag="cont")
                P.s0_b = ppool.tile([128, TILE_W], bf16, tag="s0")
                nc.scalar.copy(P.cont_b[:], P.t_con[:])
                nc.scalar.copy(P.s0_b[:], P.t_cen[:])
                oxx = ppool.tile([128, PW], bf16, tag="oxx")
                oyy = ppool.tile([128, PW], bf16, tag="oyy")
                oxy = ppool.tile([128, PW], bf16, tag="oxy")
                nc.scalar.activation(oxx[:], P.t_ox[:, WSL], Act.Square)
                nc.scalar.activation(oyy[:], P.t_oy[:, WSL], Act.Square)
                nc.gpsimd.tensor_mul(out=oxy[:], in0=P.t_ox[:, WSL],
                                     in1=P.t_oy[:, WSL])

                hxC = spool.tile([128, TILE_W - 2], bf16, tag="hx")
                nc.vector.tensor_add(out=hxC[:], in0=P.cont_b[:, 0:TILE_W - 2],
                                     in1=P.cont_b[:, 2:TILE_W])
                q1 = qpool.tile([128, PW], fp32, tag="qa")
                q2 = qpool.tile([128, PW], fp32, tag="qb")
                q3 = qpool.tile([128, PW], fp32, tag="qa")
                stencils(P.cont_b, hxC, q1, q2, q3)
                S1b = spool.tile([128, PW], bf16, tag="S1b")
                S2b = spool.tile([128, PW], bf16, tag="S2b")
                nc.scalar.copy(S1b[:], q1[:])
                nc.scalar.copy(S2b[:], q2[:])
                p1 = spool.tile([128, PW], bf16, tag="m1")
                p2 = spool.tile([128, PW], bf16, tag="m2")
                p3 = spool.tile([128, PW], bf16, tag="m3")
                nc.vector.tensor_mul(out=p1[:], in0=oxx[:], in1=S1b[:])
                nc.vector.tensor_mul(out=p2[:], in0=oyy[:], in1=S2b[:])
                nc.vector.tensor_mul(out=p3[:], in0=oxy[:], in1=q3[:])
                a1 = spool.tile([128, PW], bf16, tag="n1")
                den = fpool.tile([128, PW], fp32, tag="den")
                nc.gpsimd.tensor_add(out=a1[:], in0=p1[:], in1=p2[:])
                nc.gpsimd.tensor_add(out=den[:], in0=a1[:], in1=p3[:])
                rden = fpool.tile([128, PW], fp32, tag="rden")
                nc.vector.tensor_scalar_add(rden[:], den[:], EPS)
                nc.vector.reciprocal_approx_fast(out=rden[:], in_=rden[:])

                g4 = fpool.tile([128, PW], fp32, tag="g4")
                nc.vector.tensor_scalar(
                    out=g4[:], in0=P.t_unc[:, WSL], scalar1=1.0, scalar2=0.0,
                    op0=Alu.min, op1=Alu.max)
                nc.vector.tensor_scalar(
                    out=g4[:], in0=g4[:], scalar1=-0.25, scalar2=0.25,
                    op0=Alu.mult, op1=Alu.add)
                if P.p_lo > 0:
                    nc.vector.memset(g4[0:P.p_lo, :], 0.0)
                if P.p_hi < 128:
                    assert P.p_hi == 102
                    nc.vector.tensor_scalar(
                        out=g4[:], in0=g4[:], scalar1=botmask[:, 0:1],
                        scalar2=None, op0=Alu.mult)

                g4r = spool.tile([128, PW], bf16, tag="g4r")
                nc.gpsimd.tensor_mul(out=g4r[:], in0=g4[:], in1=rden[:])
                P.Cxx = ppool.tile([128, PW], bf16, tag="Cxx")
                P.Cyy = ppool.tile([128, PW], bf16, tag="Cyy")
                P.Cxy = ppool.tile([128, PW], bf16, tag="Cxy")
                P.G9 = ppool.tile([128, PW], bf16, tag="G9")
                nc.vector.tensor_mul(out=P.Cxx[:], in0=g4r[:], in1=oxx[:])
                nc.vector.tensor_mul(out=P.Cyy[:], in0=g4r[:], in1=oyy[:])
                nc.gpsimd.tensor_mul(out=P.Cxy[:], in0=g4r[:], in1=oxy[:])
                nc.scalar.mul(P.G9[:], g4[:], 1.0 / 9.0)
                P.s_cur = P.s0_b

            def emit_iter(P, it):
                last = it == NUM_ITERS - 1
                s_cur = P.s_cur
                cs = spool.tile([128, TILE_W], bf16, tag="cs")
                nc.gpsimd.tensor_mul(out=cs[:], in0=P.cont_b[:], in1=s_cur[:])
                hx = spool.tile([128, TILE_W - 2], bf16, tag="hx")
                nc.vector.tensor_add(out=hx[:], in0=cs[:, 0:TILE_W - 2],
                                     in1=cs[:, 2:TILE_W])
                q1 = qpool.tile([128, PW], fp32, tag="qa")
                q2 = qpool.tile([128, PW], fp32, tag="qb")
                q3 = qpool.tile([128, PW], fp32, tag="qa")
                q9 = qpool.tile([128, PW], fp32, tag="qb")
                stencils(cs, hx, q1, q2, q3)
                qsmooth(s_cur, q9)

                S1b = spool.tile([128, PW], bf16, tag="S1b")
                S2b = spool.tile([128, PW], bf16, tag="S2b")
                nc.scalar.copy(S1b[:], q1[:])
                nc.scalar.copy(S2b[:], q2[:])

                m1 = spool.tile([128, PW], bf16, tag="m1")
                m2 = spool.tile([128, PW], bf16, tag="m2")
                m3 = spool.tile([128, PW], bf16, tag="m3")
                n1 = spool.tile([128, PW], bf16, tag="n1")
                num = spool.tile([128, PW], bf16, tag="num")
                t2 = spool.tile([128, PW], bf16, tag="t2")
                sd = spool.tile([128, PW], bf16, tag="sd")
                nc.vector.tensor_mul(out=m1[:], in0=P.Cxx[:], in1=S1b[:])
                nc.vector.tensor_mul(out=m2[:], in0=P.Cyy[:], in1=S2b[:])
                nc.vector.tensor_mul(out=m3[:], in0=P.Cxy[:], in1=q3[:])
                nc.gpsimd.tensor_add(out=n1[:], in0=m1[:], in1=m2[:])
                nc.vector.tensor_mul(out=t2[:], in0=P.G9[:], in1=q9[:])
                nc.vector.tensor_add(out=num[:], in0=n1[:], in1=m3[:])
                nc.gpsimd.tensor_add(out=sd[:], in0=num[:], in1=t2[:])
                if last:
                    s_nxt = opool.tile([128, TILE_W], fp32, tag="s_f")
                else:
                    s_nxt = s_ab[it % 2]
                nc.vector.scalar_tensor_tensor(
                    out=s_nxt[:, WSL], in0=s_cur[:, WSL], scalar=0.75,
                    in1=sd[:], op0=Alu.mult, op1=Alu.add)
                P.s_cur = s_nxt

            def emit_store(P):
                nrows = P.r1 - P.r0
                nc.sync.dma_start(
                    out=out_d[P.r0:P.r1, :],
                    in_=P.s_cur[6:6 + nrows, WSL])

            for (r0, r1) in row_panels:
                P = make_panel(r0, r1)
                emit_load(P)
                emit_pre(P)
                for it in range(NUM_ITERS):
                    emit_iter(P, it)
                emit_store(P)

    nc.finalize()
    return nc


def kernel(center, continuity, orientation, uncertainty):
    from concourse.bass_utils import run_bass_kernel_spmd

    if "nc" not in _CACHE:
        _CACHE["nc"] = _build_bass()
    nc = _CACHE["nc"]

    B = center.shape[0]
    in_maps = []
    for b in range(B):
        in_maps.append({
            "center": np.ascontiguousarray(center[b, 0]),
            "continuity": np.ascontiguousarray(continuity[b, 0]),
            "orientation": np.ascontiguousarray(orientation[b]),
            "uncertainty": np.ascontiguousarray(uncertainty[b, 0]),
        })
    res = run_bass_kernel_spmd(nc, in_maps, core_ids=list(range(B)))
    out = np.stack([r["out"] for r in res.results])[:, None]
    return out.astype(np.float32)


# revision 7
# speedup vs baseline: 1.2601x; 1.2601x over previous
"""Trainium2 Bass kernel for nn_IterativeStructureRefiner (v4, bf16, full-row strips).

Math (validated vs reference: fp32 9e-8, bf16-quantized ~3e-3 l2rel):
  Rank-3 orientation factorization of the 8-neighbor affinity:
    num = oxx*S1 + oyy*S2 + oxy*S3,  S1 = Bm@hx, S2 = T0@cs + H0@hx,
    S3 = A0@csL - A0@csR   (cs = continuity*s, hx = csL+csR; vertical
    stencils are banded 128x128 stationaries on the TensorEngine).
  den = same stencils on continuity (iteration-invariant) -> folded with
  the uncertainty gate into precomputed coefficient maps:
    Cxx = 0.25*g*r*oxx (etc.), G9 = 0.25*g/9, r = 1/(den+eps).
  Smooth term in one PE pass: Q9 = T0I@sL + (T0I-9I)@sC + T0I@sR
    = 9*(box3x3(s) - s), so  s' = 0.75*s + G9*Q9 + Cxx*S1+Cyy*S2+Cxy*S3.
  The reference's final clip is provably inactive.

Geometry: 9 full-width row strips [128 part x 1024 cols] per image. Columns
have NO inter-patch halos - the only column pads are out-of-image zeros
(the reference zero-pads there), so stencil outputs are computed on exactly
1024 = 2x512 columns: every matmul chunk is 512 wide (one PSUM bank), no
tail matmuls. Rows keep the 6-row shrink-halo (116-row interior per strip).

All loop tensors bf16 (DVE 2x packing; 4B-aligned windows). PSUM rotates
two tags x two buffers. S1/S2 drain to bf16 on the Scalar engine; m3/t2
read PSUM directly on Vector; GpSimd takes cs + two adds per iteration.

Sharding: pure data-parallel, one batch image per NeuronCore (B=8).
Inputs are read once, output written once; all 6 iterations run locally.
"""

import numpy as np

H = W = 1024
TILE_W = 1040          # tile col t <-> image col t - 8 ; pads [0:8) [1032:1040)
W0, W1 = 8, 1032       # working window = the full image row (1024 cols)
PW = W1 - W0           # 1024
ROWS_OUT = 116
NUM_ITERS = 6
EPS = 1e-6

_CACHE = {}


def _build_bass():
    import concourse.bacc as bacc
    import concourse.mybir as mybir
    from concourse.tile import TileContext

    fp32 = mybir.dt.float32
    bf16 = mybir.dt.bfloat16
    Alu = mybir.AluOpType
    Act = mybir.ActivationFunctionType

    nc = bacc.Bacc("TRN2", debug=False)

    cen_d = nc.dram_tensor("center", [H, W], fp32, kind="ExternalInput")
    con_d = nc.dram_tensor("continuity", [H, W], fp32, kind="ExternalInput")
    ori_d = nc.dram_tensor("orientation", [2, H, W], fp32, kind="ExternalInput")
    unc_d = nc.dram_tensor("uncertainty", [H, W], fp32, kind="ExternalInput")
    out_d = nc.dram_tensor("out", [H, W], fp32, kind="ExternalOutput")

    k = np.arange(128)
    T0 = (np.abs(k[:, None] - k[None, :]) == 1).astype(np.float32)
    Bm = np.eye(128, dtype=np.float32) + 0.5 * T0
    H0 = 0.5 * T0
    A0 = ((k[:, None] == k[None, :] - 1).astype(np.float32)
          - (k[:, None] == k[None, :] + 1).astype(np.float32))
    A0m = -A0
    T0I = np.eye(128, dtype=np.float32) + T0
    T0I9 = T0 - 8.0 * np.eye(128, dtype=np.float32)
    ST = [Bm, T0, H0, A0, A0m, T0I, T0I9]
    st_drams = [nc.inline_tensor(m, name=f"st_{i}") for i, m in enumerate(ST)]
    botmask_np = (np.arange(128) < 102).astype(np.float32)[:, None]
    bot_dram = nc.inline_tensor(botmask_np, name="botmask")

    row_panels = []
    for r0 in range(0, H, ROWS_OUT):
        row_panels.append((r0, min(r0 + ROWS_OUT, H)))

    # psum col j <-> tile col j + W0 ; chunks are exactly one bank each
    CH = ((0, 512), (512, 1024))

    with TileContext(nc) as tc:
        with (
            tc.tile_pool(name="consts", bufs=1) as cpool,
            tc.tile_pool(name="inp", bufs=1) as ipool,
            tc.tile_pool(name="pre", bufs=1) as ppool,
            tc.tile_pool(name="scr", bufs=2) as spool,
            tc.tile_pool(name="scr2", bufs=1) as s2pool,
            tc.tile_pool(name="fpre", bufs=1) as fpool,
            tc.tile_pool(name="stg", bufs=1) as gpool,
            tc.tile_pool(name="outp", bufs=1) as opool,
            tc.tile_pool(name="psum", bufs=1, space="PSUM") as qpool,
        ):
            st_b = []
            for i, d in enumerate(st_drams):
                tf = gpool.tile([128, 128], fp32, tag=f"stf{i}")
                nc.sync.dma_start(out=tf[:], in_=d[:, :])
                tb = cpool.tile([128, 128], bf16, tag=f"st{i}")
                nc.scalar.copy(tb[:], tf[:])
                st_b.append(tb)
            tBm, tT0, tH0, tA0, tA0m, tT0I, tT0I9 = st_b
            botmask = cpool.tile([128, 1], fp32, tag="botmask")
            nc.sync.dma_start(out=botmask[:], in_=bot_dram[:, :])

            # persistent bf16 ping-pong s tiles per slot; pads zeroed once
            s_ab = {}
            for sl in (0, 1):
                pair = []
                for nm in ("s_a", "s_b"):
                    t = cpool.tile([128, TILE_W], bf16, tag=f"{nm}{sl}")
                    nc.vector.memset(t[:, 0:W0], 0.0)
                    nc.vector.memset(t[:, W1:TILE_W], 0.0)
                    pair.append(t)
                s_ab[sl] = pair

            def stencils(src, hxt, q1t, q2t, q3t):
                """q1=Bm@hx, q2=T0@src+H0@hx, q3=A0@srcL-A0@srcR.
                hxt[k] holds the hx value of tile col k+1 (shift-stored).
                Grouped per stationary to minimize LDWEIGHTS reloads."""
                for lo, hi in CH:
                    nc.tensor.matmul(q1t[:, lo:hi], tBm[:],
                                     hxt[:, W0 - 1 + lo:W0 - 1 + hi],
                                     start=True, stop=True)
                for lo, hi in CH:
                    nc.tensor.matmul(q2t[:, lo:hi], tT0[:],
                                     src[:, W0 + lo:W0 + hi],
                                     start=True, stop=False)
                for lo, hi in CH:
                    nc.tensor.matmul(q2t[:, lo:hi], tH0[:],
                                     hxt[:, W0 - 1 + lo:W0 - 1 + hi],
                                     start=False, stop=True)
                for lo, hi in CH:
                    nc.tensor.matmul(q3t[:, lo:hi], tA0[:],
                                     src[:, W0 - 1 + lo:W0 - 1 + hi],
                                     start=True, stop=False)
                for lo, hi in CH:
                    nc.tensor.matmul(q3t[:, lo:hi], tA0m[:],
                                     src[:, W0 + 1 + lo:W0 + 1 + hi],
                                     start=False, stop=True)

            def qsmooth(s_cur, q9t):
                for off in (W0 - 1, W0 + 1):
                    for lo, hi in CH:
                        nc.tensor.matmul(q9t[:, lo:hi], tT0I[:],
                                         s_cur[:, off + lo:off + hi],
                                         start=(off == W0 - 1), stop=False)
                for lo, hi in CH:
                    nc.tensor.matmul(q9t[:, lo:hi], tT0I9[:],
                                     s_cur[:, W0 + lo:W0 + hi],
                                     start=False, stop=True)

            WSL = slice(W0, W1)

            class Panel:
                pass

            def make_panel(r0, r1, sl):
                P = Panel()
                P.r0, P.r1, P.sl = r0, r1, sl
                P.row_lo = max(r0 - 6, 0)
                P.row_hi = min(r0 + 122, H)
                P.p_lo = P.row_lo - (r0 - 6)
                P.p_hi = P.row_hi - (r0 - 6)
                return P

            def emit_load(P):
                sl = P.sl

                def load(src_ap, tag):
                    t = ipool.tile([128, TILE_W], fp32, tag=f"{tag}{sl}")
                    nc.gpsimd.memset(t[:, 0:W0], 0.0)
                    nc.gpsimd.memset(t[:, W1:TILE_W], 0.0)
                    if P.p_lo > 0:
                        nc.gpsimd.memset(t[0:P.p_lo, W0:W1], 0.0)
                    if P.p_hi < 128:
                        aligned_lo = (P.p_hi // 32) * 32
                        nc.gpsimd.memset(t[aligned_lo:128, W0:W1], 0.0)
                    nc.sync.dma_start(
                        out=t[P.p_lo:P.p_hi, WSL],
                        in_=src_ap[P.row_lo:P.row_hi, :])
                    return t

                P.t_cen = load(cen_d, "cen")
                P.t_con = load(con_d, "con")
                P.t_ox = load(ori_d[0], "ox")
                P.t_oy = load(ori_d[1], "oy")
                P.t_unc = load(unc_d, "unc")

            def emit_pre(P):
                sl = P.sl
                P.cont_b = ppool.tile([128, TILE_W], bf16, tag=f"cont{sl}")
                P.s0_b = ppool.tile([128, TILE_W], bf16, tag=f"s0{sl}")
                nc.scalar.copy(P.cont_b[:], P.t_con[:])
                nc.scalar.copy(P.s0_b[:], P.t_cen[:])
                oxx = ppool.tile([128, PW], bf16, tag=f"oxx{sl}")
                oyy = ppool.tile([128, PW], bf16, tag=f"oyy{sl}")
                oxy = ppool.tile([128, PW], bf16, tag=f"oxy{sl}")
                nc.scalar.activation(oxx[:], P.t_ox[:, WSL], Act.Square)
                nc.scalar.activation(oyy[:], P.t_oy[:, WSL], Act.Square)
                nc.gpsimd.tensor_mul(out=oxy[:], in0=P.t_ox[:, WSL],
                                     in1=P.t_oy[:, WSL])

                hxC = spool.tile([128, TILE_W - 2], bf16, tag=f"hx{sl}")
                nc.vector.tensor_add(out=hxC[:], in0=P.cont_b[:, 0:TILE_W - 2],
                                     in1=P.cont_b[:, 2:TILE_W])
                q1 = qpool.tile([128, PW], fp32, tag=f"qa{sl}")
                q2 = qpool.tile([128, PW], fp32, tag=f"qb{sl}")
                q3 = qpool.tile([128, PW], fp32, tag=f"qa{sl}")
                stencils(P.cont_b, hxC, q1, q2, q3)
                S1b = spool.tile([128, PW], bf16, tag=f"S1b{sl}")
                S2b = spool.tile([128, PW], bf16, tag=f"S2b{sl}")
                nc.scalar.copy(S1b[:], q1[:])
                nc.scalar.copy(S2b[:], q2[:])
                p1 = s2pool.tile([128, PW], bf16, tag=f"m1{sl}")
                p2 = s2pool.tile([128, PW], bf16, tag=f"m2{sl}")
                p3 = s2pool.tile([128, PW], bf16, tag=f"m3{sl}")
                nc.vector.tensor_mul(out=p1[:], in0=oxx[:], in1=S1b[:])
                nc.vector.tensor_mul(out=p2[:], in0=oyy[:], in1=S2b[:])
                nc.vector.tensor_mul(out=p3[:], in0=oxy[:], in1=q3[:])
                a1 = s2pool.tile([128, PW], bf16, tag=f"n1{sl}")
                den = fpool.tile([128, PW], fp32, tag=f"fA{sl}")
                nc.gpsimd.tensor_add(out=a1[:], in0=p1[:], in1=p2[:])
                nc.gpsimd.tensor_add(out=den[:], in0=a1[:], in1=p3[:])
                rden = fpool.tile([128, PW], fp32, tag=f"fB{sl}")
                nc.vector.tensor_scalar_add(rden[:], den[:], EPS)
                nc.vector.reciprocal_approx_fast(out=rden[:], in_=rden[:])

                g4 = fpool.tile([128, PW], fp32, tag=f"fA{sl}")
                nc.vector.tensor_scalar(
                    out=g4[:], in0=P.t_unc[:, WSL], scalar1=1.0, scalar2=0.0,
                    op0=Alu.min, op1=Alu.max)
                nc.vector.tensor_scalar(
                    out=g4[:], in0=g4[:], scalar1=-0.25, scalar2=0.25,
                    op0=Alu.mult, op1=Alu.add)
                if P.p_lo > 0:
                    nc.vector.memset(g4[0:P.p_lo, :], 0.0)
                if P.p_hi < 128:
                    assert P.p_hi == 102
                    nc.vector.tensor_scalar(
                        out=g4[:], in0=g4[:], scalar1=botmask[:, 0:1],
                        scalar2=None, op0=Alu.mult)

                g4r = spool.tile([128, PW], bf16, tag=f"g4r{sl}")
                nc.gpsimd.tensor_mul(out=g4r[:], in0=g4[:], in1=rden[:])
                P.Cxx = ppool.tile([128, PW], bf16, tag=f"Cxx{sl}")
                P.Cyy = ppool.tile([128, PW], bf16, tag=f"Cyy{sl}")
                P.Cxy = ppool.tile([128, PW], bf16, tag=f"Cxy{sl}")
                P.G9 = ppool.tile([128, PW], bf16, tag=f"G9{sl}")
                nc.vector.tensor_mul(out=P.Cxx[:], in0=g4r[:], in1=oxx[:])
                nc.vector.tensor_mul(out=P.Cyy[:], in0=g4r[:], in1=oyy[:])
                nc.gpsimd.tensor_mul(out=P.Cxy[:], in0=g4r[:], in1=oxy[:])
                nc.scalar.mul(P.G9[:], g4[:], 1.0 / 9.0)
                P.s_cur = P.s0_b

            def emit_iter(P, it):
                sl = P.sl
                last = it == NUM_ITERS - 1
                s_cur = P.s_cur
                cs = spool.tile([128, TILE_W], bf16, tag=f"cs{sl}")
                nc.gpsimd.tensor_mul(out=cs[:], in0=P.cont_b[:], in1=s_cur[:])
                hx = spool.tile([128, TILE_W - 2], bf16, tag=f"hx{sl}")
                nc.vector.tensor_add(out=hx[:], in0=cs[:, 0:TILE_W - 2],
                                     in1=cs[:, 2:TILE_W])
                q1 = qpool.tile([128, PW], fp32, tag=f"qa{sl}")
                q2 = qpool.tile([128, PW], fp32, tag=f"qb{sl}")
                q3 = qpool.tile([128, PW], fp32, tag=f"qa{sl}")
                q9 = qpool.tile([128, PW], fp32, tag=f"qb{sl}")
                stencils(cs, hx, q1, q2, q3)
                qsmooth(s_cur, q9)

                S1b = spool.tile([128, PW], bf16, tag=f"S1b{sl}")
                S2b = spool.tile([128, PW], bf16, tag=f"S2b{sl}")
                nc.scalar.copy(S1b[:], q1[:])
                nc.scalar.copy(S2b[:], q2[:])

                m1 = s2pool.tile([128, PW], bf16, tag=f"m1{sl}")
                m2 = s2pool.tile([128, PW], bf16, tag=f"m2{sl}")
                m3 = s2pool.tile([128, PW], bf16, tag=f"m3{sl}")
                n1 = s2pool.tile([128, PW], bf16, tag=f"n1{sl}")
                num = s2pool.tile([128, PW], bf16, tag=f"num{sl}")
                t2 = s2pool.tile([128, PW], bf16, tag=f"t2{sl}")
                sd = s2pool.tile([128, PW], bf16, tag=f"sd{sl}")
                nc.vector.tensor_mul(out=m1[:], in0=P.Cxx[:], in1=S1b[:])
                nc.vector.tensor_mul(out=m2[:], in0=P.Cyy[:], in1=S2b[:])
                nc.vector.tensor_mul(out=m3[:], in0=P.Cxy[:], in1=q3[:])
                nc.gpsimd.tensor_add(out=n1[:], in0=m1[:], in1=m2[:])
                nc.vector.tensor_mul(out=t2[:], in0=P.G9[:], in1=q9[:])
                nc.vector.tensor_add(out=num[:], in0=n1[:], in1=m3[:])
                nc.gpsimd.tensor_add(out=sd[:], in0=num[:], in1=t2[:])
                if last:
                    s_nxt = opool.tile([128, TILE_W], fp32, tag=f"s_f{sl}")
                else:
                    s_nxt = s_ab[P.sl][it % 2]
                nc.vector.scalar_tensor_tensor(
                    out=s_nxt[:, WSL], in0=s_cur[:, WSL], scalar=0.75,
                    in1=sd[:], op0=Alu.mult, op1=Alu.add)
                P.s_cur = s_nxt

            def emit_store(P):
                nrows = P.r1 - P.r0
                nc.sync.dma_start(
                    out=out_d[P.r0:P.r1, :],
                    in_=P.s_cur[6:6 + nrows, WSL])

            for gi in range(0, len(row_panels), 2):
                group = [make_panel(*row_panels[gi + j], j)
                         for j in range(min(2, len(row_panels) - gi))]
                for P in group:
                    emit_load(P)
                for P in group:
                    emit_pre(P)
                for it in range(NUM_ITERS):
                    for P in group:
                        emit_iter(P, it)
                for P in group:
                    emit_store(P)

    nc.finalize()
    return nc


def kernel(center, continuity, orientation, uncertainty):
    from concourse.bass_utils import run_bass_kernel_spmd

    if "nc" not in _CACHE:
        _CACHE["nc"] = _build_bass()
    nc = _CACHE["nc"]

    B = center.shape[0]
    in_maps = []
    for b in range(B):
        in_maps.append({
            "center": np.ascontiguousarray(center[b, 0]),
            "continuity": np.ascontiguousarray(continuity[b, 0]),
            "orientation": np.ascontiguousarray(orientation[b]),
            "uncertainty": np.ascontiguousarray(uncertainty[b, 0]),
        })
    res = run_bass_kernel_spmd(nc, in_maps, core_ids=list(range(B)))
    out = np.stack([r["out"] for r in res.results])[:, None]
    return out.astype(np.float32)
